# revision 1
# baseline (speedup 1.0000x reference)
# KL divergence loss kernel for Trainium2 (Bass/Tile), 8-core data-parallel.
#
# Problem: KL(p||q) for diagonal Gaussians over [B=16, L=64, N=512, D=64] f32
# tensors, reduced to a scalar: mean over (B,L) of sum over (N,D) of
#   log(qs/ps) + 0.5*(ps^2 + (pm-qm)^2)/qs^2 - 0.5
#
# Strategy (pure data-parallel, hardcoded):
#   - Shard along B: core c gets B-rows [2c, 2c+2) -> [2,64,512,64], viewed as
#     [128 partitions, 32768 free] (partition = (b,l) pair, free = (n,d)).
#   - Per core, stream 16 tiles of [128, 2048] per tensor through SBUF.
#     Math is restructured to avoid division (ACT Reciprocal is blocked):
#       w  = exp(-ln(qs))            = 1/qs      (ACT, one table set: ln/exp/square)
#       r1 = ps * w, d = pm - qm, r2 = d * w     (DVE)
#       S1 = sum ln(r1) = -sum log-ratio          (ACT Ln with free accum_out)
#       S2 = sum r1^2, S3 = sum r2^2              (ACT Square with free accum_out)
#   - Each core DMAs out its [128, 3*NIT] partial-sum accumulators; the host
#     combines in float64:  mean = (-S1 + 0.5*(S2+S3))/(B*L) - N*D/2.

import numpy as np

B, L, N, D = 16, 64, 512, 64
NCORES = 8
P = 128                      # SBUF partitions = per-core B*L = (B/NCORES)*L
TOT = N * D                  # free elements per partition = 32768
F = 2048                     # tile free size
NIT = TOT // F               # iterations per core

_CACHE = {}


def build_nc():
    from contextlib import ExitStack
    import concourse.tile as tile
    from concourse import bacc, mybir

    dt = mybir.dt.float32
    AF = mybir.ActivationFunctionType

    nc = bacc.Bacc(
        "TRN2", target_bir_lowering=False, debug=False, num_devices=NCORES
    )
    pm = nc.dram_tensor("prior_mu", [P, TOT], dt, kind="ExternalInput").ap()
    ps = nc.dram_tensor("prior_sigma", [P, TOT], dt, kind="ExternalInput").ap()
    qm = nc.dram_tensor("post_mu", [P, TOT], dt, kind="ExternalInput").ap()
    qs = nc.dram_tensor("post_sigma", [P, TOT], dt, kind="ExternalInput").ap()
    out = nc.dram_tensor("acc_out", [P, 3 * NIT], dt, kind="ExternalOutput").ap()

    with tile.TileContext(nc) as tc, ExitStack() as ctx:
        io = ctx.enter_context(tc.tile_pool(name="io", bufs=3))
        accp = ctx.enter_context(tc.tile_pool(name="accp", bufs=1))
        acc = accp.tile([P, 3 * NIT], dt)

        for i in range(NIT):
            sl = np.s_[:, i * F:(i + 1) * F]
            qs_t = io.tile([P, F], dt)
            nc.sync.dma_start(qs_t[:], qs[sl])
            ps_t = io.tile([P, F], dt)
            nc.sync.dma_start(ps_t[:], ps[sl])
            pm_t = io.tile([P, F], dt)
            nc.sync.dma_start(pm_t[:], pm[sl])
            qm_t = io.tile([P, F], dt)
            nc.sync.dma_start(qm_t[:], qm[sl])

            # w = 1/qs via exp(-ln(qs)), in place in qs_t
            nc.scalar.activation(qs_t[:], qs_t[:], AF.Ln)
            nc.scalar.activation(qs_t[:], qs_t[:], AF.Exp, scale=-1.0)
            # r1 = ps*w -> ps_t ; d = pm-qm -> pm_t ; r2 = d*w -> qm_t
            nc.vector.tensor_mul(ps_t[:], ps_t[:], qs_t[:])
            nc.vector.tensor_sub(pm_t[:], pm_t[:], qm_t[:])
            nc.vector.tensor_mul(qm_t[:], pm_t[:], qs_t[:])
            # S1 += sum ln(r1)   (out overwrites pm_t, which is dead)
            nc.scalar.activation(
                pm_t[:], ps_t[:], AF.Ln, accum_out=acc[:, i:i + 1]
            )
            # S2 += sum r1^2 ; S3 += sum r2^2
            nc.scalar.activation(
                ps_t[:], ps_t[:], AF.Square, accum_out=acc[:, NIT + i:NIT + i + 1]
            )
            nc.scalar.activation(
                qm_t[:], qm_t[:], AF.Square,
                accum_out=acc[:, 2 * NIT + i:2 * NIT + i + 1],
            )

        nc.sync.dma_start(out[:], acc[:])

    nc.compile()
    return nc


def _shard(a, c):
    a = np.asarray(a, dtype=np.float32)
    return np.ascontiguousarray(a[2 * c:2 * c + 2]).reshape(P, TOT)


def make_in_maps(prior_mu, prior_sigma, post_mu, post_sigma):
    return [
        {
            "prior_mu": _shard(prior_mu, c),
            "prior_sigma": _shard(prior_sigma, c),
            "post_mu": _shard(post_mu, c),
            "post_sigma": _shard(post_sigma, c),
        }
        for c in range(NCORES)
    ]


def combine(results):
    S1 = S2 = S3 = 0.0
    for r in results:
        a = r["acc_out"].astype(np.float64)
        S1 += a[:, :NIT].sum()
        S2 += a[:, NIT:2 * NIT].sum()
        S3 += a[:, 2 * NIT:].sum()
    mean = (-S1 + 0.5 * (S2 + S3)) / (B * L) - 0.5 * N * D
    return np.float32(mean)


def kernel(prior_mu, prior_sigma, post_mu, post_sigma):
    from concourse.bass_utils import run_bass_kernel_spmd

    if "nc" not in _CACHE:
        _CACHE["nc"] = build_nc()
    nc = _CACHE["nc"]
    in_maps = make_in_maps(prior_mu, prior_sigma, post_mu, post_sigma)
    res = run_bass_kernel_spmd(nc, in_maps, list(range(NCORES)))
    return combine(res.results)


# revision 2
# speedup vs baseline: 1.0590x; 1.0590x over previous
# KL divergence loss kernel for Trainium2 (Bass/Tile), 8-core data-parallel.
#
# Problem: KL(p||q) for diagonal Gaussians over [B=16, L=64, N=512, D=64] f32
# tensors, reduced to a scalar: mean over (B,L) of sum over (N,D) of
#   log(qs/ps) + 0.5*(ps^2 + (pm-qm)^2)/qs^2 - 0.5
#
# Strategy (pure data-parallel, hardcoded):
#   - Shard along B: core c gets B-rows [2c, 2c+2) -> [2,64,512,64], viewed as
#     [128 partitions, 32768 free] (partition = (b,l) pair, free = (n,d)).
#   - Per core, stream 16 tiles of [128, 2048] per tensor through SBUF.
#     Math is restructured to avoid division (ACT Reciprocal is blocked):
#       w  = exp(-ln(qs))            = 1/qs      (ACT, one table set: ln/exp/square)
#       r1 = ps * w, d = pm - qm, r2 = d * w     (DVE)
#       S1 = sum ln(r1) = -sum log-ratio          (ACT Ln with free accum_out)
#       S2 = sum r1^2, S3 = sum r2^2              (ACT Square with free accum_out)
#   - Each core DMAs out its [128, 3*NIT] partial-sum accumulators; the host
#     combines in float64:  mean = (-S1 + 0.5*(S2+S3))/(B*L) - N*D/2.

import numpy as np

B, L, N, D = 16, 64, 512, 64
NCORES = 8
P = 128                      # SBUF partitions = per-core B*L = (B/NCORES)*L
TOT = N * D                  # free elements per partition = 32768
F = 2048                     # tile free size
NIT = TOT // F               # iterations per core

_CACHE = {}


def build_nc():
    from contextlib import ExitStack
    import concourse.tile as tile
    from concourse import bacc, mybir

    dt = mybir.dt.float32
    AF = mybir.ActivationFunctionType

    nc = bacc.Bacc(
        "TRN2", target_bir_lowering=False, debug=False, num_devices=NCORES
    )
    pm = nc.dram_tensor("prior_mu", [P, TOT], dt, kind="ExternalInput").ap()
    ps = nc.dram_tensor("prior_sigma", [P, TOT], dt, kind="ExternalInput").ap()
    qm = nc.dram_tensor("post_mu", [P, TOT], dt, kind="ExternalInput").ap()
    qs = nc.dram_tensor("post_sigma", [P, TOT], dt, kind="ExternalInput").ap()
    out = nc.dram_tensor("acc_out", [P, 3 * NIT], dt, kind="ExternalOutput").ap()

    with tile.TileContext(nc) as tc, ExitStack() as ctx:
        io = ctx.enter_context(tc.tile_pool(name="io", bufs=3))
        accp = ctx.enter_context(tc.tile_pool(name="accp", bufs=1))
        acc = accp.tile([P, 3 * NIT], dt)
        # ACT needs a full-size out even when only accum_out matters; park it
        # in one scratch tile (WAW on ACT only — sequential there anyway).
        scr = accp.tile([P, F], dt)

        for i in range(NIT):
            sl = np.s_[:, i * F:(i + 1) * F]
            qs_t = io.tile([P, F], dt)
            nc.sync.dma_start(qs_t[:], qs[sl])
            ps_t = io.tile([P, F], dt)
            nc.sync.dma_start(ps_t[:], ps[sl])
            pm_t = io.tile([P, F], dt)
            nc.sync.dma_start(pm_t[:], pm[sl])
            qm_t = io.tile([P, F], dt)
            nc.sync.dma_start(qm_t[:], qm[sl])

            # w = 1/qs, in place in qs_t (single custom-DVE op, ~51 ULP)
            nc.vector.reciprocal_approx_fast(out=qs_t[:], in_=qs_t[:])
            # d = pm - qm on the otherwise-idle GpSimd engine
            nc.gpsimd.tensor_sub(pm_t[:], pm_t[:], qm_t[:])
            # r1 = ps*w -> ps_t ; r2 = d*w -> qm_t
            nc.vector.tensor_mul(ps_t[:], ps_t[:], qs_t[:])
            nc.vector.tensor_mul(qm_t[:], pm_t[:], qs_t[:])
            # S1 += sum ln(r1) ; S2 += sum r1^2 ; S3 += sum r2^2
            # (Ln and Square share one ACT table set -> single table load)
            nc.scalar.activation(
                scr[:], ps_t[:], AF.Ln, accum_out=acc[:, i:i + 1]
            )
            nc.scalar.activation(
                scr[:], ps_t[:], AF.Square, accum_out=acc[:, NIT + i:NIT + i + 1]
            )
            nc.scalar.activation(
                scr[:], qm_t[:], AF.Square,
                accum_out=acc[:, 2 * NIT + i:2 * NIT + i + 1],
            )

        nc.sync.dma_start(out[:], acc[:])

    nc.compile()
    return nc


def _shard(a, c):
    a = np.asarray(a, dtype=np.float32)
    return np.ascontiguousarray(a[2 * c:2 * c + 2]).reshape(P, TOT)


def make_in_maps(prior_mu, prior_sigma, post_mu, post_sigma):
    return [
        {
            "prior_mu": _shard(prior_mu, c),
            "prior_sigma": _shard(prior_sigma, c),
            "post_mu": _shard(post_mu, c),
            "post_sigma": _shard(post_sigma, c),
        }
        for c in range(NCORES)
    ]


def combine(results):
    S1 = S2 = S3 = 0.0
    for r in results:
        a = r["acc_out"].astype(np.float64)
        S1 += a[:, :NIT].sum()
        S2 += a[:, NIT:2 * NIT].sum()
        S3 += a[:, 2 * NIT:].sum()
    mean = (-S1 + 0.5 * (S2 + S3)) / (B * L) - 0.5 * N * D
    return np.float32(mean)


def kernel(prior_mu, prior_sigma, post_mu, post_sigma):
    from concourse.bass_utils import run_bass_kernel_spmd

    if "nc" not in _CACHE:
        _CACHE["nc"] = build_nc()
    nc = _CACHE["nc"]
    in_maps = make_in_maps(prior_mu, prior_sigma, post_mu, post_sigma)
    res = run_bass_kernel_spmd(nc, in_maps, list(range(NCORES)))
    return combine(res.results)
